# revision 5
# baseline (speedup 1.0000x reference)
"""Trainium2 Bass kernel for nn_DGraphAttention (gnn_message_passing).

Math (reference):
    x = hidden_states.reshape(N, H)
    q/k/v = x @ W{q,k,v}.T + b
    src, tgt = sort(edges_src), sort(edges_tgt)        # [E] each
    scores = softmax((q[tgt] @ k[src].T) / sqrt(HEAD), axis=0)   # over tgt axis
    v[tgt] = scores @ v[src]
    return v.reshape(B, S, H)

Sharding (8 cores):
  - node rows split 4096/core for the V linear (data-parallel, weights replicated)
  - tgt rows of the E x E score matrix split 1024/core
  - x[src] is gathered on host and replicated; each core recomputes k[src], v[src]
    (8.6 GFLOP/core) which is far cheaper than all-gathering 32MB via collectives
  - softmax normalizer (per-src-column sum over the sharded tgt axis) is the only
    cross-core communication: one AllReduce of a [128, 64] f32 buffer
  - exp-scores (32MB/core) spill to DRAM between the normalizer pass and the
    output matmul; v[src] rows are rescaled by 1/colsum instead of rescaling e

All matmuls run as float32r (full fp32 data; 1 cycle/row on PE for free dim>=256).
"""

import os
import sys

sys.path.insert(0, "/opt/trn_rl_repo")

import numpy as np
from contextlib import ExitStack

import concourse.bass as bass
import concourse.bacc as bacc
import concourse.mybir as mybir
from concourse.tile import TileContext
from concourse.tile_rust import add_dep_helper
from concourse.bass_utils import run_bass_kernel_spmd

F32 = mybir.dt.float32
F32R = mybir.dt.float32r
AF = mybir.ActivationFunctionType

# problem constants
N_CORES = 8
B, S, H, NH = 4, 8192, 512, 8
HEAD = H // NH          # 64
N = B * S               # 32768
E = 8192
P = 128
FREE = 512              # matmul moving free dim (fp32 max, = 1 psum bank)

N_OWN = N // N_CORES    # 4096 node rows per core
N_TGT = E // N_CORES    # 1024 tgt score rows per core

LAST_RESULT = None      # BassKernelResults of the most recent run (for test harness)
_PROGRAM = None


def _r(x):
    return x


def build_program(h=H, e=E, n_own=N_OWN, n_tgt=N_TGT, n_cores=N_CORES, jblk=512):
    """Build the SPMD Bass program. All sizes in elements; h % 128 == 0,
    e % jblk == 0, jblk % 128 == 0, n_own % FREE == 0."""
    ft_n = h // P           # feature tiles
    jt_n = e // P           # src row tiles
    njb = e // jblk         # j blocks in the A/B loop
    j4_n = jblk // P        # 128-row tiles per j block
    ic_n = max(1, n_tgt // FREE)   # i chunks (tgt) per matmul pass
    icf = min(FREE, n_tgt)         # i chunk free size
    oc_n = max(1, n_own // FREE)
    ocf = min(FREE, n_own)
    inv_sqrt_head = 1.0 / np.sqrt(HEAD)

    nc = bacc.Bacc(num_devices=n_cores)

    xT_own = nc.declare_dram_parameter("xT_own", [h, n_own], F32R, isOutput=False)
    xT_src = nc.declare_dram_parameter("xT_src", [h, e], F32R, isOutput=False)
    xT_tgt = nc.declare_dram_parameter("xT_tgt", [h, n_tgt], F32R, isOutput=False)
    wqT = nc.declare_dram_parameter("wqT", [h, h], F32R, isOutput=False)
    wkT = nc.declare_dram_parameter("wkT", [h, h], F32R, isOutput=False)
    wvT = nc.declare_dram_parameter("wvT", [h, h], F32R, isOutput=False)
    bq_t = nc.declare_dram_parameter("bq_t", [P, ft_n], F32, isOutput=False)
    bk_t = nc.declare_dram_parameter("bk_t", [P, ft_n], F32, isOutput=False)
    bv_bc = nc.declare_dram_parameter("bv_bc", [P, h], F32, isOutput=False)
    v_own = nc.declare_dram_parameter("v_own", [n_own, h], F32, isOutput=True)
    outT_tgt = nc.declare_dram_parameter("outT_tgt", [h, n_tgt], F32, isOutput=True)

    cc_in = nc.dram_tensor("cc_in", [P, jt_n], F32)
    cc_out = nc.dram_tensor("cc_out", [P, jt_n], F32, addr_space="Shared")

    with TileContext(nc) as tc, ExitStack() as ctx:
        persist = ctx.enter_context(tc.tile_pool(name="persist", bufs=1))
        dram = ctx.enter_context(tc.tile_pool(name="dram", bufs=1, space="DRAM"))

        # persistent SBUF state
        wq_sb = persist.tile([P, ft_n, h], F32R)
        nc.sync.dma_start(wq_sb[:], wqT.rearrange("(ft p) f -> p ft f", p=P))
        wk_sb = persist.tile([P, ft_n, h], F32R)
        nc.sync.dma_start(wk_sb[:], wkT.rearrange("(ft p) f -> p ft f", p=P))
        wv_sb = persist.tile([P, ft_n, h], F32R)
        nc.sync.dma_start(wv_sb[:], wvT.rearrange("(ft p) f -> p ft f", p=P))
        bq_sb = persist.tile([P, ft_n], F32)
        nc.sync.dma_start(bq_sb[:], bq_t[:])
        bk_sb = persist.tile([P, ft_n], F32)
        nc.sync.dma_start(bk_sb[:], bk_t[:])
        bvb_sb = persist.tile([P, h], F32)
        nc.sync.dma_start(bvb_sb[:], bv_bc[:])
        xtg_sb = persist.tile([P, ft_n, n_tgt], F32R)
        nc.sync.dma_start(xtg_sb[:], xT_tgt.rearrange("(ft p) i -> p ft i", p=P))
        q_sb = persist.tile([P, ft_n, n_tgt], F32R)
        colsum_sb = persist.tile([P, jt_n], F32)
        csg_sb = persist.tile([P, jt_n], F32)
        recip_sb = persist.tile([P, jt_n], F32)

        # DRAM spill buffers
        e_dram = dram.tile([jt_n, P, n_tgt], F32R)
        vs_dram = dram.tile([jt_n, P, h], F32R)

        # ---- phase Q: q_tgt^T = Wq^T-matmul + bias, [h, n_tgt] f-major ----
        with tc.tile_pool(name="psq", bufs=2, space="PSUM") as psq:
            for ftile in range(ft_n):
                for ic in range(ic_n):
                    pq = psq.tile([P, icf], F32)
                    for fs in range(ft_n):
                        nc.tensor.matmul(
                            pq[:],
                            _r(wq_sb[:, fs, ftile * P:(ftile + 1) * P]),
                            _r(xtg_sb[:, fs, ic * icf:(ic + 1) * icf]),
                            start=(fs == 0), stop=(fs == ft_n - 1),
                        )
                    nc.scalar.activation(
                        q_sb[:, ftile, ic * icf:(ic + 1) * icf], pq[:],
                        AF.Identity, bias=bq_sb[:, ftile:ftile + 1],
                    )

        # ---- A/B loop: k_src^T, v_src, exp-scores + colsum, spill ----
        with (
            tc.tile_pool(name="xs", bufs=3) as xsp,
            tc.tile_pool(name="ks", bufs=2) as ksp,
            tc.tile_pool(name="et", bufs=3) as etp,
            tc.tile_pool(name="vsb", bufs=3) as vsp,
            tc.tile_pool(name="psk", bufs=2, space="PSUM") as psk,
            tc.tile_pool(name="psv", bufs=2, space="PSUM") as psv,
            tc.tile_pool(name="pss", bufs=2, space="PSUM") as pss,
        ):
            for jb in range(njb):
                xs = xsp.tile([P, ft_n, jblk], F32R)
                nc.sync.dma_start(
                    xs[:],
                    xT_src[:, jb * jblk:(jb + 1) * jblk].rearrange(
                        "(fs p) j -> p fs j", p=P),
                )
                # k_src^T block: [h(f-major), jblk]
                ks = ksp.tile([P, ft_n, jblk], F32R)
                for ftile in range(ft_n):
                    pk = psk.tile([P, jblk], F32)
                    for fs in range(ft_n):
                        nc.tensor.matmul(
                            pk[:],
                            _r(wk_sb[:, fs, ftile * P:(ftile + 1) * P]),
                            _r(xs[:, fs, :]),
                            start=(fs == 0), stop=(fs == ft_n - 1),
                        )
                    nc.scalar.activation(
                        ks[:, ftile, :], pk[:], AF.Identity,
                        bias=bk_sb[:, ftile:ftile + 1],
                    )
                # v_src block: [jblk(j-major), h], spilled to DRAM
                for j4 in range(j4_n):
                    jt = jb * j4_n + j4
                    pv = psv.tile([P, h], F32)
                    for fs in range(ft_n):
                        nc.tensor.matmul(
                            pv[:],
                            _r(xs[:, fs, j4 * P:(j4 + 1) * P]),
                            _r(wv_sb[:, fs, :]),
                            start=(fs == 0), stop=(fs == ft_n - 1),
                        )
                    vt = vsp.tile([P, h], F32R)
                    nc.vector.tensor_add(vt[:], pv[:], bvb_sb[:])
                    nc.sync.dma_start(vs_dram[jt], vt[:])
                # scores^T block: e^T[jblk, n_tgt] = exp(s/8), colsum via accum_out
                for j4 in range(j4_n):
                    jt = jb * j4_n + j4
                    ps = pss.tile([P, n_tgt], F32)
                    for ftile in range(ft_n):
                        for ic in range(ic_n):
                            nc.tensor.matmul(
                                ps[:, ic * icf:(ic + 1) * icf],
                                _r(ks[:, ftile, j4 * P:(j4 + 1) * P]),
                                _r(q_sb[:, ftile, ic * icf:(ic + 1) * icf]),
                                start=(ftile == 0), stop=(ftile == ft_n - 1),
                            )
                    et = etp.tile([P, n_tgt], F32R)
                    nc.scalar.activation(
                        et[:], ps[:], AF.Exp, scale=float(inv_sqrt_head),
                        accum_out=colsum_sb[:, jt:jt + 1],
                    )
                    nc.sync.dma_start(e_dram[jt], et[:])

        # ---- colsum AllReduce across the 8 cores ----
        d1 = nc.sync.dma_start(cc_in[:], colsum_sb[:])
        cc = nc.gpsimd.collective_compute(
            "AllReduce",
            mybir.AluOpType.add,
            replica_groups=[list(range(n_cores))],
            ins=[cc_in[:]],
            outs=[cc_out[:]],
        )
        add_dep_helper(cc.ins, d1.ins, sync=True,
                       reason="colsum store before allreduce")
        d2 = nc.sync.dma_start(csg_sb[:], cc_out[:])
        add_dep_helper(d2.ins, cc.ins, sync=True,
                       reason="allreduce before readback")
        nc.vector.reciprocal(recip_sb[:], csg_sb[:])

        # ---- phase E: v_own = x_own @ Wv.T + bv (overlaps the collective) ----
        with (
            tc.tile_pool(name="xo", bufs=3) as xop,
            tc.tile_pool(name="vo", bufs=3) as vop,
            tc.tile_pool(name="pse", bufs=2, space="PSUM") as pse,
        ):
            v_own_t = v_own.rearrange("(ot p) f -> ot p f", p=P)
            for oc in range(oc_n):
                xo = xop.tile([P, ft_n, ocf], F32R)
                nc.sync.dma_start(
                    xo[:],
                    xT_own[:, oc * ocf:(oc + 1) * ocf].rearrange(
                        "(fs p) o -> p fs o", p=P),
                )
                for o4 in range(ocf // P):
                    pe_ = pse.tile([P, h], F32)
                    for fs in range(ft_n):
                        nc.tensor.matmul(
                            pe_[:],
                            _r(xo[:, fs, o4 * P:(o4 + 1) * P]),
                            _r(wv_sb[:, fs, :]),
                            start=(fs == 0), stop=(fs == ft_n - 1),
                        )
                    vo = vop.tile([P, h], F32)
                    nc.vector.tensor_add(vo[:], pe_[:], bvb_sb[:])
                    nc.sync.dma_start(v_own_t[oc * (ocf // P) + o4], vo[:])

        # ---- phase C/D: out^T = (v_src/colsum)^T-matmul over spilled e ----
        with (
            tc.tile_pool(name="ce", bufs=4) as cep,
            tc.tile_pool(name="cv", bufs=4) as cvp,
            tc.tile_pool(name="co", bufs=2) as cop,
            tc.tile_pool(name="psc", bufs=1, space="PSUM") as pscp,
        ):
            psc = pscp.tile([P, ft_n, n_tgt], F32)
            for jt in range(jt_n):
                et = cep.tile([P, n_tgt], F32R)
                nc.sync.dma_start(et[:], e_dram[jt])
                vt = cvp.tile([P, h], F32R)
                nc.sync.dma_start(vt[:], vs_dram[jt])
                nc.vector.tensor_scalar_mul(vt[:], vt[:], recip_sb[:, jt:jt + 1])
                for ftile in range(ft_n):
                    for ic in range(ic_n):
                        nc.tensor.matmul(
                            psc[:, ftile, ic * icf:(ic + 1) * icf],
                            _r(vt[:, ftile * P:(ftile + 1) * P]),
                            _r(et[:, ic * icf:(ic + 1) * icf]),
                            start=(jt == 0), stop=(jt == jt_n - 1),
                        )
            for ftile in range(ft_n):
                ot = cop.tile([P, n_tgt], F32)
                nc.vector.tensor_copy(ot[:], psc[:, ftile, :])
                nc.sync.dma_start(outT_tgt[ftile * P:(ftile + 1) * P, :], ot[:])

    nc.compile()
    return nc


def _get_program():
    global _PROGRAM
    if _PROGRAM is None:
        _PROGRAM = build_program()
    return _PROGRAM


def make_in_maps(hidden_states, Wq, bq, Wk, bk, Wv, bv, edges_src, edges_tgt,
                 h=H, e=E, n_own=N_OWN, n_tgt=N_TGT, n_cores=N_CORES):
    """Host-side sharding: sort indices, gather rows, transpose to f-major."""
    ft_n = h // P
    n = n_own * n_cores
    x = np.ascontiguousarray(
        np.asarray(hidden_states, dtype=np.float32).reshape(n, h))
    src = np.sort(np.asarray(edges_src).astype(np.int64))
    tgt = np.sort(np.asarray(edges_tgt).astype(np.int64))
    xT = np.ascontiguousarray(x.T)                      # [h, n]
    xT_src = np.ascontiguousarray(xT[:, src])           # [h, e]
    wqT = np.ascontiguousarray(np.asarray(Wq, np.float32).T)
    wkT = np.ascontiguousarray(np.asarray(Wk, np.float32).T)
    wvT = np.ascontiguousarray(np.asarray(Wv, np.float32).T)
    bq_t = np.ascontiguousarray(np.asarray(bq, np.float32).reshape(ft_n, P).T)
    bk_t = np.ascontiguousarray(np.asarray(bk, np.float32).reshape(ft_n, P).T)
    bv_bc = np.ascontiguousarray(
        np.tile(np.asarray(bv, np.float32)[None, :], (P, 1)))
    in_maps = []
    for c in range(n_cores):
        in_maps.append({
            "xT_own": np.ascontiguousarray(xT[:, c * n_own:(c + 1) * n_own]),
            "xT_src": xT_src,
            "xT_tgt": np.ascontiguousarray(
                xT[:, tgt[c * n_tgt:(c + 1) * n_tgt]]),
            "wqT": wqT, "wkT": wkT, "wvT": wvT,
            "bq_t": bq_t, "bk_t": bk_t, "bv_bc": bv_bc,
        })
    return in_maps, tgt


def assemble_output(results, tgt, h=H, n_own=N_OWN, n_tgt=N_TGT,
                    n_cores=N_CORES, out_shape=(B, S, H)):
    n = n_own * n_cores
    v = np.empty((n, h), np.float32)
    for c in range(n_cores):
        v[c * n_own:(c + 1) * n_own] = results[c]["v_own"]
    outs = np.concatenate(
        [results[c]["outT_tgt"].T for c in range(n_cores)], axis=0)
    v[tgt] = outs
    return v.reshape(out_shape)


def kernel(hidden_states, Wq, bq, Wk, bk, Wv, bv, edges_src, edges_tgt):
    global LAST_RESULT
    in_maps, tgt = make_in_maps(
        hidden_states, Wq, bq, Wk, bk, Wv, bv, edges_src, edges_tgt)
    nc = _get_program()
    res = run_bass_kernel_spmd(nc, in_maps, list(range(N_CORES)))
    LAST_RESULT = res
    return assemble_output(res.results, tgt)


# revision 7
# speedup vs baseline: 1.0555x; 1.0555x over previous
"""Trainium2 Bass kernel for nn_DGraphAttention (gnn_message_passing).

Math (reference):
    x = hidden_states.reshape(N, H)
    q/k/v = x @ W{q,k,v}.T + b
    src, tgt = sort(edges_src), sort(edges_tgt)        # [E] each
    scores = softmax((q[tgt] @ k[src].T) / sqrt(HEAD), axis=0)   # over tgt axis
    v[tgt] = scores @ v[src]
    return v.reshape(B, S, H)

Sharding (8 cores):
  - node rows split 4096/core for the V linear (data-parallel, weights replicated)
  - tgt rows of the E x E score matrix split 1024/core
  - x[src] is gathered on host and replicated; each core recomputes k[src], v[src]
    (8.6 GFLOP/core) which is far cheaper than all-gathering 32MB via collectives
  - softmax normalizer (per-src-column sum over the sharded tgt axis) is the only
    cross-core communication: one AllReduce of a [128, 64] f32 buffer
  - exp-scores (32MB/core) spill to DRAM between the normalizer pass and the
    output matmul; v[src] rows are rescaled by 1/colsum instead of rescaling e

All matmuls run as float32r (full fp32 data; 1 cycle/row on PE for free dim>=256).
"""

import os
import sys

sys.path.insert(0, "/opt/trn_rl_repo")

import numpy as np
from contextlib import ExitStack

import concourse.bass as bass
import concourse.bacc as bacc
import concourse.mybir as mybir
from concourse.tile import TileContext
from concourse.tile_rust import add_dep_helper
from concourse.bass_utils import run_bass_kernel_spmd

F32 = mybir.dt.float32
F32R = mybir.dt.float32r
BF16 = mybir.dt.bfloat16
AF = mybir.ActivationFunctionType

# problem constants
N_CORES = 8
B, S, H, NH = 4, 8192, 512, 8
HEAD = H // NH          # 64
N = B * S               # 32768
E = 8192
P = 128
FREE = 512              # matmul moving free dim (fp32 max, = 1 psum bank)

N_OWN = N // N_CORES    # 4096 node rows per core
N_TGT = E // N_CORES    # 1024 tgt score rows per core

LAST_RESULT = None      # BassKernelResults of the most recent run (for test harness)
_PROGRAM = None


def _r(x):
    return x


def build_program(h=H, e=E, n_own=N_OWN, n_tgt=N_TGT, n_cores=N_CORES, jblk=512,
                  sc_dt=None):
    """Build the SPMD Bass program. All sizes in elements; h % 128 == 0,
    e % jblk == 0, jblk % 128 == 0, n_own % FREE == 0."""
    ft_n = h // P           # feature tiles
    jt_n = e // P           # src row tiles
    njb = e // jblk         # j blocks in the A/B loop
    j4_n = jblk // P        # 128-row tiles per j block
    ic_n = max(1, n_tgt // FREE)   # i chunks (tgt) per matmul pass
    icf = min(FREE, n_tgt)         # i chunk free size
    oc_n = max(1, n_own // FREE)
    ocf = min(FREE, n_own)
    inv_sqrt_head = 1.0 / np.sqrt(HEAD)
    if sc_dt is None:
        sc_dt = F32R   # scores-path dtype: F32R (accurate) or BF16 (fast)

    nc = bacc.Bacc(num_devices=n_cores)

    xT_own = nc.declare_dram_parameter("xT_own", [h, n_own], F32R, isOutput=False)
    xT_src = nc.declare_dram_parameter("xT_src", [h, e], F32R, isOutput=False)
    xT_tgt = nc.declare_dram_parameter("xT_tgt", [h, n_tgt], F32R, isOutput=False)
    wqT = nc.declare_dram_parameter("wqT", [h, h], F32R, isOutput=False)
    wkT = nc.declare_dram_parameter("wkT", [h, h], F32R, isOutput=False)
    wvT = nc.declare_dram_parameter("wvT", [h, h], F32R, isOutput=False)
    bq_t = nc.declare_dram_parameter("bq_t", [P, ft_n], F32, isOutput=False)
    bk_t = nc.declare_dram_parameter("bk_t", [P, ft_n], F32, isOutput=False)
    bv_bc = nc.declare_dram_parameter("bv_bc", [P, h], F32, isOutput=False)
    v_own = nc.declare_dram_parameter("v_own", [n_own, h], F32, isOutput=True)
    outT_tgt = nc.declare_dram_parameter("outT_tgt", [h, n_tgt], F32, isOutput=True)

    cc_in = nc.dram_tensor("cc_in", [P, jt_n], F32)
    cc_out = nc.dram_tensor("cc_out", [P, jt_n], F32, addr_space="Shared")

    with TileContext(nc) as tc, ExitStack() as ctx:
        persist = ctx.enter_context(tc.tile_pool(name="persist", bufs=1))
        dram = ctx.enter_context(tc.tile_pool(name="dram", bufs=1, space="DRAM"))

        # persistent SBUF state; phase Q critical-path loads (wq, xtg, bq)
        # are issued first so the first matmul starts ASAP
        wq_sb = persist.tile([P, ft_n, h], F32R)
        nc.sync.dma_start(wq_sb[:], wqT.rearrange("(ft p) f -> p ft f", p=P))
        xtg_sb = persist.tile([P, ft_n, n_tgt], F32R)
        nc.sync.dma_start(xtg_sb[:], xT_tgt.rearrange("(ft p) i -> p ft i", p=P))
        bq_sb = persist.tile([P, ft_n], F32)
        nc.sync.dma_start(bq_sb[:], bq_t[:])
        wk_sb = persist.tile([P, ft_n, h], F32R)
        nc.sync.dma_start(wk_sb[:], wkT.rearrange("(ft p) f -> p ft f", p=P))
        wv_sb = persist.tile([P, ft_n, h], F32R)
        nc.sync.dma_start(wv_sb[:], wvT.rearrange("(ft p) f -> p ft f", p=P))
        bk_sb = persist.tile([P, ft_n], F32)
        nc.sync.dma_start(bk_sb[:], bk_t[:])
        bvb_sb = persist.tile([P, h], F32)
        nc.sync.dma_start(bvb_sb[:], bv_bc[:])
        q_sb = persist.tile([P, ft_n, n_tgt], sc_dt)
        colsum_sb = persist.tile([P, jt_n], F32)
        csg_sb = persist.tile([P, jt_n], F32)
        recip_sb = persist.tile([P, jt_n], F32)

        # DRAM spill buffers
        e_dram = dram.tile([jt_n, P, n_tgt], sc_dt)
        vs_dram = dram.tile([jt_n, P, h], sc_dt)

        # ---- phase Q: q_tgt^T = Wq^T-matmul + bias, [h, n_tgt] f-major ----
        with tc.tile_pool(name="psq", bufs=2, space="PSUM") as psq:
            for ftile in range(ft_n):
                for ic in range(ic_n):
                    pq = psq.tile([P, icf], F32)
                    for fs in range(ft_n):
                        nc.tensor.matmul(
                            pq[:],
                            _r(wq_sb[:, fs, ftile * P:(ftile + 1) * P]),
                            _r(xtg_sb[:, fs, ic * icf:(ic + 1) * icf]),
                            start=(fs == 0), stop=(fs == ft_n - 1),
                        )
                    nc.scalar.activation(
                        q_sb[:, ftile, ic * icf:(ic + 1) * icf], pq[:],
                        AF.Identity, bias=bq_sb[:, ftile:ftile + 1],
                    )

        # ---- A/B loop: k_src^T, v_src, exp-scores + colsum, spill ----
        # xT_own chunks for phase E prefetch during the A/B loop (DMA slack)
        xop = ctx.enter_context(tc.tile_pool(name="xo", bufs=oc_n))
        xo_tiles = []
        with (
            tc.tile_pool(name="xs", bufs=3) as xsp,
            tc.tile_pool(name="ks", bufs=2) as ksp,
            tc.tile_pool(name="et", bufs=4) as etp,
            tc.tile_pool(name="vsb", bufs=4) as vsp,
            tc.tile_pool(name="psk", bufs=2, space="PSUM") as psk,
            tc.tile_pool(name="psv", bufs=2, space="PSUM") as psv,
            tc.tile_pool(name="pss", bufs=2, space="PSUM") as pss,
        ):
            for jb in range(njb):
                oc = jb * oc_n // njb
                if oc * njb == jb * oc_n:   # spread the oc_n prefetches evenly
                    xo = xop.tile([P, ft_n, ocf], F32R)
                    nc.sync.dma_start(
                        xo[:],
                        xT_own[:, oc * ocf:(oc + 1) * ocf].rearrange(
                            "(fs p) o -> p fs o", p=P),
                    )
                    xo_tiles.append(xo)
                xs = xsp.tile([P, ft_n, jblk], F32R)
                nc.sync.dma_start(
                    xs[:],
                    xT_src[:, jb * jblk:(jb + 1) * jblk].rearrange(
                        "(fs p) j -> p fs j", p=P),
                )
                # k_src^T block: [h(f-major), jblk]
                ks = ksp.tile([P, ft_n, jblk], sc_dt)
                for ftile in range(ft_n):
                    pk = psk.tile([P, jblk], F32)
                    for fs in range(ft_n):
                        nc.tensor.matmul(
                            pk[:],
                            _r(wk_sb[:, fs, ftile * P:(ftile + 1) * P]),
                            _r(xs[:, fs, :]),
                            start=(fs == 0), stop=(fs == ft_n - 1),
                        )
                    nc.scalar.activation(
                        ks[:, ftile, :], pk[:], AF.Identity,
                        bias=bk_sb[:, ftile:ftile + 1],
                    )
                # v_src block: [jblk(j-major), h], spilled to DRAM
                for j4 in range(j4_n):
                    jt = jb * j4_n + j4
                    pv = psv.tile([P, h], F32)
                    for fs in range(ft_n):
                        nc.tensor.matmul(
                            pv[:],
                            _r(xs[:, fs, j4 * P:(j4 + 1) * P]),
                            _r(wv_sb[:, fs, :]),
                            start=(fs == 0), stop=(fs == ft_n - 1),
                        )
                    vt = vsp.tile([P, h], sc_dt)
                    nc.vector.tensor_add(vt[:], pv[:], bvb_sb[:])
                    nc.sync.dma_start(vs_dram[jt], vt[:])
                # scores^T block: e^T[jblk, n_tgt] = exp(s/8), colsum via accum_out
                for j4 in range(j4_n):
                    jt = jb * j4_n + j4
                    ps = pss.tile([P, n_tgt], F32)
                    for ftile in range(ft_n):
                        for ic in range(ic_n):
                            nc.tensor.matmul(
                                ps[:, ic * icf:(ic + 1) * icf],
                                _r(ks[:, ftile, j4 * P:(j4 + 1) * P]),
                                _r(q_sb[:, ftile, ic * icf:(ic + 1) * icf]),
                                start=(ftile == 0), stop=(ftile == ft_n - 1),
                            )
                    et = etp.tile([P, n_tgt], sc_dt)
                    nc.scalar.activation(
                        et[:], ps[:], AF.Exp, scale=float(inv_sqrt_head),
                        accum_out=colsum_sb[:, jt:jt + 1],
                    )
                    nc.sync.dma_start(e_dram[jt], et[:])

        # ---- colsum AllReduce across the 8 cores ----
        d1 = nc.sync.dma_start(cc_in[:], colsum_sb[:])
        cc = nc.gpsimd.collective_compute(
            "AllReduce",
            mybir.AluOpType.add,
            replica_groups=[list(range(n_cores))],
            ins=[cc_in[:]],
            outs=[cc_out[:]],
        )
        add_dep_helper(cc.ins, d1.ins, sync=True,
                       reason="colsum store before allreduce")
        d2 = nc.sync.dma_start(csg_sb[:], cc_out[:])
        add_dep_helper(d2.ins, cc.ins, sync=True,
                       reason="allreduce before readback")
        nc.vector.reciprocal(recip_sb[:], csg_sb[:])

        # ---- phase E: v_own = x_own @ Wv.T + bv (overlaps the collective) ----
        with (
            tc.tile_pool(name="vo", bufs=3) as vop,
            tc.tile_pool(name="pse", bufs=2, space="PSUM") as pse,
        ):
            v_own_t = v_own.rearrange("(ot p) f -> ot p f", p=P)
            for oc in range(oc_n):
                xo = xo_tiles[oc]
                for o4 in range(ocf // P):
                    pe_ = pse.tile([P, h], F32)
                    for fs in range(ft_n):
                        nc.tensor.matmul(
                            pe_[:],
                            _r(xo[:, fs, o4 * P:(o4 + 1) * P]),
                            _r(wv_sb[:, fs, :]),
                            start=(fs == 0), stop=(fs == ft_n - 1),
                        )
                    vo = vop.tile([P, h], F32)
                    nc.vector.tensor_add(vo[:], pe_[:], bvb_sb[:])
                    nc.sync.dma_start(v_own_t[oc * (ocf // P) + o4], vo[:])

        # ---- phase C/D: out^T = (v_src/colsum)^T-matmul over spilled e ----
        with (
            tc.tile_pool(name="ce", bufs=6) as cep,
            tc.tile_pool(name="cv", bufs=6) as cvp,
            tc.tile_pool(name="co", bufs=2) as cop,
            tc.tile_pool(name="psc", bufs=1, space="PSUM") as pscp,
        ):
            psc = pscp.tile([P, ft_n, n_tgt], F32)
            for jt in range(jt_n):
                et = cep.tile([P, n_tgt], sc_dt)
                nc.sync.dma_start(et[:], e_dram[jt])
                vt = cvp.tile([P, h], sc_dt)
                nc.sync.dma_start(vt[:], vs_dram[jt])
                nc.vector.tensor_scalar_mul(vt[:], vt[:], recip_sb[:, jt:jt + 1])
                for ftile in range(ft_n):
                    for ic in range(ic_n):
                        nc.tensor.matmul(
                            psc[:, ftile, ic * icf:(ic + 1) * icf],
                            _r(vt[:, ftile * P:(ftile + 1) * P]),
                            _r(et[:, ic * icf:(ic + 1) * icf]),
                            start=(jt == 0), stop=(jt == jt_n - 1),
                        )
            for ftile in range(ft_n):
                ot = cop.tile([P, n_tgt], F32)
                nc.vector.tensor_copy(ot[:], psc[:, ftile, :])
                nc.sync.dma_start(outT_tgt[ftile * P:(ftile + 1) * P, :], ot[:])

    nc.compile()
    return nc


def _get_program():
    global _PROGRAM
    if _PROGRAM is None:
        sc = BF16 if os.environ.get("DGA_SCORES_BF16") == "1" else F32R
        _PROGRAM = build_program(sc_dt=sc)
    return _PROGRAM


def make_in_maps(hidden_states, Wq, bq, Wk, bk, Wv, bv, edges_src, edges_tgt,
                 h=H, e=E, n_own=N_OWN, n_tgt=N_TGT, n_cores=N_CORES):
    """Host-side sharding: sort indices, gather rows, transpose to f-major."""
    ft_n = h // P
    n = n_own * n_cores
    x = np.ascontiguousarray(
        np.asarray(hidden_states, dtype=np.float32).reshape(n, h))
    src = np.sort(np.asarray(edges_src).astype(np.int64))
    tgt = np.sort(np.asarray(edges_tgt).astype(np.int64))
    xT = np.ascontiguousarray(x.T)                      # [h, n]
    xT_src = np.ascontiguousarray(xT[:, src])           # [h, e]
    wqT = np.ascontiguousarray(np.asarray(Wq, np.float32).T)
    wkT = np.ascontiguousarray(np.asarray(Wk, np.float32).T)
    wvT = np.ascontiguousarray(np.asarray(Wv, np.float32).T)
    bq_t = np.ascontiguousarray(np.asarray(bq, np.float32).reshape(ft_n, P).T)
    bk_t = np.ascontiguousarray(np.asarray(bk, np.float32).reshape(ft_n, P).T)
    bv_bc = np.ascontiguousarray(
        np.tile(np.asarray(bv, np.float32)[None, :], (P, 1)))
    in_maps = []
    for c in range(n_cores):
        in_maps.append({
            "xT_own": np.ascontiguousarray(xT[:, c * n_own:(c + 1) * n_own]),
            "xT_src": xT_src,
            "xT_tgt": np.ascontiguousarray(
                xT[:, tgt[c * n_tgt:(c + 1) * n_tgt]]),
            "wqT": wqT, "wkT": wkT, "wvT": wvT,
            "bq_t": bq_t, "bk_t": bk_t, "bv_bc": bv_bc,
        })
    return in_maps, tgt


def assemble_output(results, tgt, h=H, n_own=N_OWN, n_tgt=N_TGT,
                    n_cores=N_CORES, out_shape=(B, S, H)):
    n = n_own * n_cores
    v = np.empty((n, h), np.float32)
    for c in range(n_cores):
        v[c * n_own:(c + 1) * n_own] = results[c]["v_own"]
    outs = np.concatenate(
        [results[c]["outT_tgt"].T for c in range(n_cores)], axis=0)
    v[tgt] = outs
    return v.reshape(out_shape)


def kernel(hidden_states, Wq, bq, Wk, bk, Wv, bv, edges_src, edges_tgt):
    global LAST_RESULT
    in_maps, tgt = make_in_maps(
        hidden_states, Wq, bq, Wk, bk, Wv, bv, edges_src, edges_tgt)
    nc = _get_program()
    res = run_bass_kernel_spmd(nc, in_maps, list(range(N_CORES)))
    LAST_RESULT = res
    return assemble_output(res.results, tgt)


# revision 10
# speedup vs baseline: 1.1283x; 1.0690x over previous
"""Trainium2 Bass kernel for nn_DGraphAttention (gnn_message_passing).

Math (reference):
    x = hidden_states.reshape(N, H)
    q/k/v = x @ W{q,k,v}.T + b
    src, tgt = sort(edges_src), sort(edges_tgt)        # [E] each
    scores = softmax((q[tgt] @ k[src].T) / sqrt(HEAD), axis=0)   # over tgt axis
    v[tgt] = scores @ v[src]
    return v.reshape(B, S, H)

Sharding (8 cores):
  - node rows split 4096/core for the V linear (data-parallel, weights replicated)
  - tgt rows of the E x E score matrix split 1024/core
  - x[src] is gathered on host and replicated; each core recomputes k[src], v[src]
    (8.6 GFLOP/core) which is far cheaper than all-gathering 32MB via collectives
  - softmax normalizer (per-src-column sum over the sharded tgt axis) is the only
    cross-core communication: one AllReduce of a [128, 64] f32 buffer
  - exp-scores (32MB/core) spill to DRAM between the normalizer pass and the
    output matmul; v[src] rows are rescaled by 1/colsum instead of rescaling e

All matmuls run as float32r (full fp32 data; 1 cycle/row on PE for free dim>=256).
"""

import os
import sys

sys.path.insert(0, "/opt/trn_rl_repo")

import numpy as np
from contextlib import ExitStack

import concourse.bass as bass
import concourse.bacc as bacc
import concourse.mybir as mybir
from concourse.tile import TileContext
from concourse.tile_rust import add_dep_helper
from concourse.bass_utils import run_bass_kernel_spmd

F32 = mybir.dt.float32
F32R = mybir.dt.float32r
BF16 = mybir.dt.bfloat16
AF = mybir.ActivationFunctionType

# problem constants
N_CORES = 8
B, S, H, NH = 4, 8192, 512, 8
HEAD = H // NH          # 64
N = B * S               # 32768
E = 8192
P = 128
FREE = 512              # matmul moving free dim (fp32 max, = 1 psum bank)

N_OWN = N // N_CORES    # 4096 node rows per core
N_TGT = E // N_CORES    # 1024 tgt score rows per core

LAST_RESULT = None      # BassKernelResults of the most recent run (for test harness)
_PROGRAM = None


def _r(x):
    return x


def build_program(h=H, e=E, n_own=N_OWN, n_tgt=N_TGT, n_cores=N_CORES, jblk=512,
                  sc_dt=None):
    """Build the SPMD Bass program. All sizes in elements; h % 128 == 0,
    e % jblk == 0, jblk % 128 == 0, n_own % FREE == 0."""
    ft_n = h // P           # feature tiles
    jt_n = e // P           # src row tiles
    njb = e // jblk         # j blocks in the A/B loop
    j4_n = jblk // P        # 128-row tiles per j block
    ic_n = max(1, n_tgt // FREE)   # i chunks (tgt) per matmul pass
    icf = min(FREE, n_tgt)         # i chunk free size
    oc_n = max(1, n_own // FREE)
    ocf = min(FREE, n_own)
    inv_sqrt_head = 1.0 / np.sqrt(HEAD)
    if sc_dt is None:
        sc_dt = F32R   # scores-path dtype: F32R (accurate) or BF16 (fast)

    nc = bacc.Bacc(num_devices=n_cores)

    xT_own = nc.declare_dram_parameter("xT_own", [h, n_own], F32R, isOutput=False)
    xT_src = nc.declare_dram_parameter("xT_src", [h, e], F32R, isOutput=False)
    xT_tgt = nc.declare_dram_parameter("xT_tgt", [h, n_tgt], F32R, isOutput=False)
    wqT = nc.declare_dram_parameter("wqT", [h, h], F32R, isOutput=False)
    wkT = nc.declare_dram_parameter("wkT", [h, h], F32R, isOutput=False)
    wvT = nc.declare_dram_parameter("wvT", [h, h], F32R, isOutput=False)
    bq_t = nc.declare_dram_parameter("bq_t", [P, ft_n], F32, isOutput=False)
    bk_t = nc.declare_dram_parameter("bk_t", [P, ft_n], F32, isOutput=False)
    bv_bc = nc.declare_dram_parameter("bv_bc", [P, h], F32, isOutput=False)
    v_own = nc.declare_dram_parameter("v_own", [n_own, h], F32, isOutput=True)
    outT_tgt = nc.declare_dram_parameter("outT_tgt", [h, n_tgt], F32, isOutput=True)

    cc_in_a = nc.dram_tensor("cc_in_a", [P, jt_n // 2], F32)
    cc_out_a = nc.dram_tensor("cc_out_a", [P, jt_n // 2], F32, addr_space="Shared")
    cc_in_b = nc.dram_tensor("cc_in_b", [P, jt_n - jt_n // 2], F32)
    cc_out_b = nc.dram_tensor("cc_out_b", [P, jt_n - jt_n // 2], F32, addr_space="Shared")

    with TileContext(nc) as tc, ExitStack() as ctx:
        persist = ctx.enter_context(tc.tile_pool(name="persist", bufs=1))
        dram = ctx.enter_context(tc.tile_pool(name="dram", bufs=1, space="DRAM"))

        # persistent SBUF state; phase Q critical-path loads (wq, xtg, bq)
        # are issued first so the first matmul starts ASAP
        wq_sb = persist.tile([P, ft_n, h], F32R)
        nc.sync.dma_start(wq_sb[:], wqT.rearrange("(ft p) f -> p ft f", p=P))
        xtg_sb = persist.tile([P, ft_n, n_tgt], F32R)
        nc.sync.dma_start(xtg_sb[:], xT_tgt.rearrange("(ft p) i -> p ft i", p=P))
        bq_sb = persist.tile([P, ft_n], F32)
        nc.sync.dma_start(bq_sb[:], bq_t[:])
        wk_sb = persist.tile([P, ft_n, h], F32R)
        nc.sync.dma_start(wk_sb[:], wkT.rearrange("(ft p) f -> p ft f", p=P))
        wv_sb = persist.tile([P, ft_n, h], F32R)
        nc.sync.dma_start(wv_sb[:], wvT.rearrange("(ft p) f -> p ft f", p=P))
        bk_sb = persist.tile([P, ft_n], F32)
        nc.sync.dma_start(bk_sb[:], bk_t[:])
        bvb_sb = persist.tile([P, h], F32)
        nc.sync.dma_start(bvb_sb[:], bv_bc[:])
        q_sb = persist.tile([P, ft_n, n_tgt], sc_dt)
        jt_half = jt_n // 2
        colsum_a = persist.tile([P, jt_half], F32)
        colsum_b = persist.tile([P, jt_n - jt_half], F32)
        csg_sb = persist.tile([P, jt_n], F32)
        recip_sb = persist.tile([P, jt_n], F32)

        # DRAM spill buffers
        e_dram = dram.tile([jt_n, P, n_tgt], sc_dt)
        vs_dram = dram.tile([jt_n, P, h], sc_dt)

        # ---- phase Q: q_tgt^T = Wq^T-matmul + bias, [h, n_tgt] f-major ----
        with tc.tile_pool(name="psq", bufs=2, space="PSUM") as psq:
            for ftile in range(ft_n):
                for ic in range(ic_n):
                    pq = psq.tile([P, icf], F32)
                    for fs in range(ft_n):
                        nc.tensor.matmul(
                            pq[:],
                            _r(wq_sb[:, fs, ftile * P:(ftile + 1) * P]),
                            _r(xtg_sb[:, fs, ic * icf:(ic + 1) * icf]),
                            start=(fs == 0), stop=(fs == ft_n - 1),
                        )
                    nc.scalar.activation(
                        q_sb[:, ftile, ic * icf:(ic + 1) * icf], pq[:],
                        AF.Identity, bias=bq_sb[:, ftile:ftile + 1],
                    )

        # ---- A/B loop: k_src^T, v_src, exp-scores + colsum, spill ----
        # xT_own chunks for phase E prefetch during the A/B loop (DMA slack)
        xop = ctx.enter_context(tc.tile_pool(name="xo", bufs=oc_n))
        xo_tiles = []
        with (
            tc.tile_pool(name="xs", bufs=3) as xsp,
            tc.tile_pool(name="ks", bufs=2) as ksp,
            tc.tile_pool(name="et", bufs=4) as etp,
            tc.tile_pool(name="vsb", bufs=4) as vsp,
            tc.tile_pool(name="psk", bufs=2, space="PSUM") as psk,
            tc.tile_pool(name="psv", bufs=2, space="PSUM") as psv,
            tc.tile_pool(name="pss", bufs=2, space="PSUM") as pss,
        ):
            for jb in range(njb):
                oc = jb * oc_n // njb
                if oc * njb == jb * oc_n:   # spread the oc_n prefetches evenly
                    xo = xop.tile([P, ft_n, ocf], F32R)
                    nc.sync.dma_start(
                        xo[:],
                        xT_own[:, oc * ocf:(oc + 1) * ocf].rearrange(
                            "(fs p) o -> p fs o", p=P),
                    )
                    xo_tiles.append(xo)
                xs = xsp.tile([P, ft_n, jblk], F32R)
                nc.sync.dma_start(
                    xs[:],
                    xT_src[:, jb * jblk:(jb + 1) * jblk].rearrange(
                        "(fs p) j -> p fs j", p=P),
                )
                # k_src^T block: [h(f-major), jblk]
                ks = ksp.tile([P, ft_n, jblk], sc_dt)
                for ftile in range(ft_n):
                    pk = psk.tile([P, jblk], F32)
                    for fs in range(ft_n):
                        nc.tensor.matmul(
                            pk[:],
                            _r(wk_sb[:, fs, ftile * P:(ftile + 1) * P]),
                            _r(xs[:, fs, :]),
                            start=(fs == 0), stop=(fs == ft_n - 1),
                        )
                    nc.scalar.activation(
                        ks[:, ftile, :], pk[:], AF.Identity,
                        bias=bk_sb[:, ftile:ftile + 1],
                    )
                # v_src block: [jblk(j-major), h], spilled to DRAM
                for j4 in range(j4_n):
                    jt = jb * j4_n + j4
                    pv = psv.tile([P, h], F32)
                    for fs in range(ft_n):
                        nc.tensor.matmul(
                            pv[:],
                            _r(xs[:, fs, j4 * P:(j4 + 1) * P]),
                            _r(wv_sb[:, fs, :]),
                            start=(fs == 0), stop=(fs == ft_n - 1),
                        )
                    vt = vsp.tile([P, h], sc_dt)
                    nc.vector.tensor_add(vt[:], pv[:], bvb_sb[:])
                    nc.sync.dma_start(vs_dram[jt], vt[:])
                # scores^T block: e^T[jblk, n_tgt] = exp(s/8), colsum via accum_out
                for j4 in range(j4_n):
                    jt = jb * j4_n + j4
                    ps = pss.tile([P, n_tgt], F32)
                    for ftile in range(ft_n):
                        for ic in range(ic_n):
                            nc.tensor.matmul(
                                ps[:, ic * icf:(ic + 1) * icf],
                                _r(ks[:, ftile, j4 * P:(j4 + 1) * P]),
                                _r(q_sb[:, ftile, ic * icf:(ic + 1) * icf]),
                                start=(ftile == 0), stop=(ftile == ft_n - 1),
                            )
                    et = etp.tile([P, n_tgt], sc_dt)
                    if jt < jt_half:
                        acc = colsum_a[:, jt:jt + 1]
                    else:
                        acc = colsum_b[:, jt - jt_half:jt - jt_half + 1]
                    nc.scalar.activation(
                        et[:], ps[:], AF.Exp, scale=float(inv_sqrt_head),
                        accum_out=acc,
                    )
                    nc.sync.dma_start(e_dram[jt], et[:])

                if jb == max(njb // 2 - 1, 0):
                    # first-half colsum AllReduce, hidden under remaining A/B work
                    d1a = nc.sync.dma_start(cc_in_a[:], colsum_a[:])
                    cca = nc.gpsimd.collective_compute(
                        "AllReduce", mybir.AluOpType.add,
                        replica_groups=[list(range(n_cores))],
                        ins=[cc_in_a[:]], outs=[cc_out_a[:]],
                    )
                    add_dep_helper(cca.ins, d1a.ins, sync=True,
                                   reason="colsum_a store before allreduce")
                    d2a = nc.sync.dma_start(csg_sb[:, :jt_half], cc_out_a[:])
                    add_dep_helper(d2a.ins, cca.ins, sync=True,
                                   reason="allreduce_a before readback")
                    nc.vector.reciprocal(recip_sb[:, :jt_half],
                                         csg_sb[:, :jt_half])

        # ---- second-half colsum AllReduce ----
        d1b = nc.sync.dma_start(cc_in_b[:], colsum_b[:])
        ccb = nc.gpsimd.collective_compute(
            "AllReduce", mybir.AluOpType.add,
            replica_groups=[list(range(n_cores))],
            ins=[cc_in_b[:]], outs=[cc_out_b[:]],
        )
        add_dep_helper(ccb.ins, d1b.ins, sync=True,
                       reason="colsum_b store before allreduce")
        d2b = nc.sync.dma_start(csg_sb[:, jt_half:], cc_out_b[:])
        add_dep_helper(d2b.ins, ccb.ins, sync=True,
                       reason="allreduce_b before readback")
        nc.vector.reciprocal(recip_sb[:, jt_half:], csg_sb[:, jt_half:])

        # ---- phase E: v_own = x_own @ Wv.T + bv (overlaps the collective) ----
        with (
            tc.tile_pool(name="vo", bufs=3) as vop,
            tc.tile_pool(name="pse", bufs=2, space="PSUM") as pse,
        ):
            v_own_t = v_own.rearrange("(ot p) f -> ot p f", p=P)
            for oc in range(oc_n):
                xo = xo_tiles[oc]
                for o4 in range(ocf // P):
                    pe_ = pse.tile([P, h], F32)
                    for fs in range(ft_n):
                        nc.tensor.matmul(
                            pe_[:],
                            _r(xo[:, fs, o4 * P:(o4 + 1) * P]),
                            _r(wv_sb[:, fs, :]),
                            start=(fs == 0), stop=(fs == ft_n - 1),
                        )
                    vo = vop.tile([P, h], F32)
                    nc.vector.tensor_add(vo[:], pe_[:], bvb_sb[:])
                    nc.sync.dma_start(v_own_t[oc * (ocf // P) + o4], vo[:])

        # ---- phase C/D: out^T = (v_src/colsum)^T-matmul over spilled e ----
        with (
            tc.tile_pool(name="ce", bufs=6) as cep,
            tc.tile_pool(name="cv", bufs=6) as cvp,
            tc.tile_pool(name="co", bufs=2) as cop,
            tc.tile_pool(name="psc", bufs=1, space="PSUM") as pscp,
        ):
            psc_f = [pscp.tile([P, n_tgt], F32, tag=f"psc{f}",
                               name=f"psc{f}")
                     for f in range(ft_n)]
            for jt in range(jt_n):
                et = cep.tile([P, n_tgt], sc_dt)
                nc.sync.dma_start(et[:], e_dram[jt])
                vt = cvp.tile([P, h], sc_dt)
                nc.sync.dma_start(vt[:], vs_dram[jt])
                nc.vector.tensor_scalar_mul(vt[:], vt[:], recip_sb[:, jt:jt + 1])
                for ftile in range(ft_n):
                    for ic in range(ic_n):
                        nc.tensor.matmul(
                            psc_f[ftile][:, ic * icf:(ic + 1) * icf],
                            _r(vt[:, ftile * P:(ftile + 1) * P]),
                            _r(et[:, ic * icf:(ic + 1) * icf]),
                            start=(jt == 0), stop=(jt == jt_n - 1),
                        )
            for ftile in range(ft_n):
                ot = cop.tile([P, n_tgt], F32)
                nc.vector.tensor_copy(ot[:], psc_f[ftile][:])
                nc.sync.dma_start(outT_tgt[ftile * P:(ftile + 1) * P, :], ot[:])

    nc.compile()
    return nc


def _get_program():
    global _PROGRAM
    if _PROGRAM is None:
        sc = BF16 if os.environ.get("DGA_SCORES_BF16") == "1" else F32R
        _PROGRAM = build_program(sc_dt=sc)
    return _PROGRAM


def make_in_maps(hidden_states, Wq, bq, Wk, bk, Wv, bv, edges_src, edges_tgt,
                 h=H, e=E, n_own=N_OWN, n_tgt=N_TGT, n_cores=N_CORES):
    """Host-side sharding: sort indices, gather rows, transpose to f-major."""
    ft_n = h // P
    n = n_own * n_cores
    x = np.ascontiguousarray(
        np.asarray(hidden_states, dtype=np.float32).reshape(n, h))
    src = np.sort(np.asarray(edges_src).astype(np.int64))
    tgt = np.sort(np.asarray(edges_tgt).astype(np.int64))
    xT = np.ascontiguousarray(x.T)                      # [h, n]
    xT_src = np.ascontiguousarray(xT[:, src])           # [h, e]
    wqT = np.ascontiguousarray(np.asarray(Wq, np.float32).T)
    wkT = np.ascontiguousarray(np.asarray(Wk, np.float32).T)
    wvT = np.ascontiguousarray(np.asarray(Wv, np.float32).T)
    bq_t = np.ascontiguousarray(np.asarray(bq, np.float32).reshape(ft_n, P).T)
    bk_t = np.ascontiguousarray(np.asarray(bk, np.float32).reshape(ft_n, P).T)
    bv_bc = np.ascontiguousarray(
        np.tile(np.asarray(bv, np.float32)[None, :], (P, 1)))
    in_maps = []
    for c in range(n_cores):
        in_maps.append({
            "xT_own": np.ascontiguousarray(xT[:, c * n_own:(c + 1) * n_own]),
            "xT_src": xT_src,
            "xT_tgt": np.ascontiguousarray(
                xT[:, tgt[c * n_tgt:(c + 1) * n_tgt]]),
            "wqT": wqT, "wkT": wkT, "wvT": wvT,
            "bq_t": bq_t, "bk_t": bk_t, "bv_bc": bv_bc,
        })
    return in_maps, tgt


def assemble_output(results, tgt, h=H, n_own=N_OWN, n_tgt=N_TGT,
                    n_cores=N_CORES, out_shape=(B, S, H)):
    n = n_own * n_cores
    v = np.empty((n, h), np.float32)
    for c in range(n_cores):
        v[c * n_own:(c + 1) * n_own] = results[c]["v_own"]
    outs = np.concatenate(
        [results[c]["outT_tgt"].T for c in range(n_cores)], axis=0)
    v[tgt] = outs
    return v.reshape(out_shape)


def kernel(hidden_states, Wq, bq, Wk, bk, Wv, bv, edges_src, edges_tgt):
    global LAST_RESULT
    in_maps, tgt = make_in_maps(
        hidden_states, Wq, bq, Wk, bk, Wv, bv, edges_src, edges_tgt)
    nc = _get_program()
    res = run_bass_kernel_spmd(nc, in_maps, list(range(N_CORES)))
    LAST_RESULT = res
    return assemble_output(res.results, tgt)
